# revision 1
# baseline (speedup 1.0000x reference)
"""Trainium2 Bass kernel for nn_MultiHeadLocalAttention (B=2,S=2048,W=32,D=1024,H=16).

Sharding: query-chunk parallel over 8 cores. Core c handles query rows
s' in [c*256, (c+1)*256) for both batches. Because of the reference's raw
.view on the k/v projections, head h of query s' reads k rows
s = h*128 + s'//16, w = 2*(s' mod 16) + w'//16 — i.e. core c needs exactly
k/v rows {h*128 + c*16 + j : h in [0,16), j in [0,16)}, giving a perfect
8-way split of the dominant k/v projection GEMMs (34 GFLOP each per core).

Per-core device row order for k/v is (b, h, p, j, i) where the original
(s, w) maps as s = h*128 + c*16 + j, w = 2i + p. With that order:
  - scores row (p,j,i) needs q_proj row u = 128*jh + 16j' + i (j = 8jh+j'),
    which is the SAME partition index in the matching q tile — no gather.
  - softmax over h is a free-dim reduce over per-(b,h) score planes.
  - the p-pair sum of the attention-weighted v partials is a PSUM
    accumulation of two masked matmuls that simultaneously produce the
    transposed layout needed by the output projection.

All matmuls run in bf16 (fp32 accumulate in PSUM). The 1/sqrt(hd) scale is
folded into Wq/bq on the host.
"""
import sys
import types

sys.path.insert(0, "/opt/trn_rl_repo")

import numpy as np
import ml_dtypes

import concourse.bass as bass
import concourse.mybir as mybir
import concourse.tile as tile
from concourse.bass_utils import run_bass_kernel_spmd

BF16 = mybir.dt.bfloat16
F32 = mybir.dt.float32
NPBF = ml_dtypes.bfloat16

B, S, W, D, H = 2, 2048, 32, 1024, 16
P = 128
NCORES = 8
ROWS = B * H * 2 * 16 * 16  # 16384 k/v rows per core, order (b,h,p,j,i)
NBH = B * H  # 32 (b,h) blocks of 512 rows


class _TC(tile.TileContext):
    """Walrus in this env rejects instructions carrying >1 sem wait (its
    setupSyncWait has a single wait slot). Two fixes at context exit:
    1. split any multi-wait instruction: excess waits move to same-engine
       NOPs inserted immediately before it (per-engine stream order makes
       this equivalent);
    2. emit the exit-drain's waits as individual SP wait_ge instructions
       instead of on the Drain itself."""

    def _split_multi_waits(self):
        nc = self.nc
        eng = {
            mybir.EngineType.PE: nc.tensor,
            mybir.EngineType.DVE: nc.vector,
            mybir.EngineType.Activation: nc.scalar,
            mybir.EngineType.Pool: nc.gpsimd,
            mybir.EngineType.SP: nc.sync,
        }
        end_bb = nc.cur_bb.bb
        for f in nc.m.functions:
            for blk in f.blocks:
                snapshot = list(blk.instructions)
                inserts = {}
                created = []
                for idx, ins in enumerate(snapshot):
                    si = getattr(ins, "sync_info", None)
                    if si is None or not si.on_wait or len(si.on_wait) <= 1:
                        continue
                    waits = list(si.on_wait)
                    nops = []
                    for w in waits[:-1]:
                        assert w.wait_reg is None, "register wait can't be split"
                        b = eng[ins.engine].nop()  # appends to end_bb
                        b.ins.sync_info = mybir.SyncInfo(on_wait=[w], on_update=[])
                        nops.append(b.ins)
                        created.append(b.ins)
                    si.on_wait = waits[-1:]
                    inserts[idx] = nops
                if not inserts:
                    continue
                created_ids = {id(n) for n in created}
                # pull the freshly-appended nops back out of the end block
                end_bb.instructions = [
                    i for i in end_bb.instructions if id(i) not in created_ids
                ]
                out = []
                for idx, ins in enumerate(snapshot):
                    out.extend(inserts.get(idx, ()))
                    out.append(ins)
                blk.instructions = out

    def _drain_and_barrier(self, tick_clock, wait_clock):
        self._split_multi_waits()
        gc = tick_clock.global_clock
        for proc, sem in sorted(wait_clock.sems.allocated().items()):
            ticks = gc.peek_next(proc) - 1
            if ticks > 0:
                val = ticks * (16 if sem.name.startswith("DMA") else 1)
                self.nc.sync.wait_ge(sem, val)
        self.nc.sync.drain()
        self.nc.all_engine_barrier()
        popped = self.nc._tile_sem_poison_stack.pop()
        assert popped is self._sem_poison
        self.nc.clear_and_free_semaphores(list(self.sems.allocated().values()))
        self.nc.all_engine_barrier()


def _build_nc(with_bias=True):
    import contextlib

    nc = bass.Bass()
    kTb = nc.dram_tensor("kTb", [P, NBH, 8, 512], BF16, kind="ExternalInput")
    vTb = nc.dram_tensor("vTb", [P, NBH, 8, 512], BF16, kind="ExternalInput")
    qTb = nc.dram_tensor("qTb", [P, 8, 512], BF16, kind="ExternalInput")
    wqd = nc.dram_tensor("wq", [P, 8, D], BF16, kind="ExternalInput")
    wkd = nc.dram_tensor("wk", [P, 8, D], BF16, kind="ExternalInput")
    wvd = nc.dram_tensor("wv", [P, 8, D], BF16, kind="ExternalInput")
    wod = nc.dram_tensor("wo", [P, 8, D], BF16, kind="ExternalInput")
    bqd = nc.dram_tensor("bq", [D], F32, kind="ExternalInput")
    bkd = nc.dram_tensor("bk", [D], F32, kind="ExternalInput")
    bvd = nc.dram_tensor("bv", [D], F32, kind="ExternalInput")
    bod = nc.dram_tensor("bo", [D], F32, kind="ExternalInput")
    pmaskd = nc.dram_tensor("pmask", [P, P], BF16, kind="ExternalInput")
    seld = nc.dram_tensor("sel", [P, 64], BF16, kind="ExternalInput")
    outd = nc.dram_tensor("out", [512, D], F32, kind="ExternalOutput")

    with _TC(nc) as tc, contextlib.ExitStack() as ex:
        wpool = ex.enter_context(tc.tile_pool(name="wts", bufs=1))
        persist = ex.enter_context(tc.tile_pool(name="persist", bufs=1))
        blk = ex.enter_context(tc.tile_pool(name="blk", bufs=3))
        actp = ex.enter_context(tc.tile_pool(name="act", bufs=10))
        prodp = ex.enter_context(tc.tile_pool(name="prod", bufs=4))
        smallp = ex.enter_context(tc.tile_pool(name="small", bufs=10))
        gps = ex.enter_context(tc.tile_pool(name="gps", bufs=6 if not with_bias else 4,
                                            space="PSUM"))
        pps = ex.enter_context(tc.tile_pool(name="pps", bufs=2, space="PSUM"))

        # ---- constants -------------------------------------------------
        def load_w(dram_t, name):
            t = wpool.tile([P, 8, D], BF16, name=name)
            nc.sync.dma_start(t[:], dram_t[:])
            return t

        wq = load_w(wqd, "wq")
        wk = load_w(wkd, "wk")
        wv = load_w(wvd, "wv")
        wo = load_w(wod, "wo")

        def load_b(dram_t, name):
            t = wpool.tile([P, D], F32, name=name)
            nc.sync.dma_start(t[:], dram_t[:].partition_broadcast(P))
            return t

        if with_bias:
            bq = load_b(bqd, "bq")
            bk = load_b(bkd, "bk")
            bv = load_b(bvd, "bv")
            bo = load_b(bod, "bo")

        pmask = wpool.tile([P, P], BF16, name="pmask")
        nc.sync.dma_start(pmask[:], pmaskd[:])
        sel = wpool.tile([P, 64], BF16, name="sel")
        nc.sync.dma_start(sel[:], seld[:])

        # ---- q projection ---------------------------------------------
        qts = persist.tile([P, 8, 512], BF16, name="qts")
        nc.sync.dma_start(qts[:], qTb[:])
        qp = persist.tile([P, 4, D], BF16, name="qp")  # u-chunks (b, jh)
        for t in range(4):
            for oh in range(2):
                ps = gps.tile([P, 512], F32, name="gps")
                for fc in range(8):
                    nc.tensor.matmul(
                        ps[:],
                        qts[:, fc, 128 * t:128 * t + 128],
                        wq[:, fc, 512 * oh:512 * oh + 512],
                        start=(fc == 0),
                        stop=(fc == 7),
                    )
                if with_bias:
                    nc.vector.tensor_add(
                        qp[:, t, 512 * oh:512 * oh + 512],
                        ps[:],
                        bq[:, 512 * oh:512 * oh + 512],
                    )
                else:
                    nc.scalar.copy(qp[:, t, 512 * oh:512 * oh + 512], ps[:])

        # ---- score planes / attn planes (persistent) -------------------
        planes = [[persist.tile([P, H, 16], F32, name=f"pl_{b}_{t}")
                   for t in range(4)] for b in range(B)]
        attns = [[persist.tile([P, H, 16], BF16, name=f"at_{b}_{t}")
                  for t in range(4)] for b in range(B)]
        # transposed out_local, chunked by contraction block g and row-chunk:
        # olT[rc][cl, g, r] = out_local[row 128*rc + r, c = 128*g + cl]
        olTs = [persist.tile([P, 8, P], BF16, name=f"olT{rc}")
                for rc in range(4)]

        def proj_block(src_dram, bh, w_s, bias):
            """Load one (b,h) 512-row block and run its projection GEMM.
            Yields, per row-tile t (row order (p, jh)), a list of
            (ap_view_c_e, c0, nc_c) sources: each a [P, nc_c, 64] view
            (c-major) covering feature blocks [c0, c0+nc_c)."""
            ts_ = blk.tile([P, 8, 512], BF16, name="blk")
            nc.sync.dma_start(ts_[:], src_dram[:, bh])
            outs = []
            for t in range(4):
                if with_bias:
                    kv = actp.tile([P, D], BF16, name="kv")
                halves = []
                for oh in range(2):
                    ps = gps.tile([P, 512], F32, name="gps")
                    for fc in range(8):
                        nc.tensor.matmul(
                            ps[:],
                            ts_[:, fc, 128 * t:128 * t + 128],
                            w_s[:, fc, 512 * oh:512 * oh + 512],
                            start=(fc == 0),
                            stop=(fc == 7),
                        )
                    if with_bias:
                        nc.vector.tensor_add(
                            kv[:, 512 * oh:512 * oh + 512],
                            ps[:],
                            bias[:, 512 * oh:512 * oh + 512],
                        )
                    else:
                        halves.append(ps)
                if with_bias:
                    outs.append([(kv[:].rearrange("p (c e) -> p c e", c=16),
                                  0, 16)])
                else:
                    outs.append([
                        (halves[0][:].rearrange("p (c e) -> p c e", c=8), 0, 8),
                        (halves[1][:].rearrange("p (c e) -> p c e", c=8), 8, 8),
                    ])
            return outs

        # ---- K phase: projections + scores -----------------------------
        for b in range(B):
            for h in range(H):
                bh = b * H + h
                kps = proj_block(kTb, bh, wk, bk if with_bias else None)
                for t in range(4):
                    jh = t % 2
                    qsm = (qp[:, 2 * b + jh, 64 * h:64 * h + 64][:, None, :])
                    prod = prodp.tile([P, 16, 64], BF16, name="prod")
                    for src, c0, ncc in kps[t]:
                        nc.vector.tensor_mul(
                            prod[:, c0:c0 + ncc, :],
                            src,
                            qsm.broadcast_to([P, ncc, 64]),
                        )
                    nc.vector.tensor_reduce(
                        planes[b][t][:, h, :],
                        prod[:],
                        axis=mybir.AxisListType.X,
                        op=mybir.AluOpType.add,
                    )

        # ---- softmax over heads ----------------------------------------
        for b in range(B):
            for t in range(4):
                pl = planes[b][t]
                mx = smallp.tile([P, 16], F32, name="mx")
                nc.vector.tensor_reduce(
                    mx[:], pl[:].rearrange("p h c -> p c h"),
                    axis=mybir.AxisListType.X, op=mybir.AluOpType.max,
                )
                sub = prodp.tile([P, H, 16], F32, name="sm")
                nc.vector.tensor_sub(
                    sub[:], pl[:],
                    mx[:][:, None, :].broadcast_to([P, H, 16]),
                )
                epl = prodp.tile([P, H, 16], F32, name="ep")
                nc.scalar.activation(
                    epl[:], sub[:], mybir.ActivationFunctionType.Exp,
                )
                z = smallp.tile([P, 16], F32, name="z")
                nc.vector.tensor_reduce(
                    z[:], epl[:].rearrange("p h c -> p c h"),
                    axis=mybir.AxisListType.X, op=mybir.AluOpType.add,
                )
                rz = smallp.tile([P, 16], F32, name="rz")
                nc.vector.reciprocal(rz[:], z[:])
                nc.vector.tensor_mul(
                    attns[b][t][:], epl[:],
                    rz[:][:, None, :].broadcast_to([P, H, 16]),
                )

        # ---- V phase: projections + weighted sum + transpose-repack ----
        for b in range(B):
            for h in range(H):
                bh = b * H + h
                vps = proj_block(vTb, bh, wv, bv if with_bias else None)
                masked = []
                for t in range(4):
                    prod2 = prodp.tile([P, 64, 16], BF16, name="pr2")
                    for src, c0, ncc in vps[t]:
                        nc.vector.tensor_mul(
                            prod2[:, :, c0:c0 + ncc],
                            src.rearrange("p c e -> p e c"),
                            attns[b][t][:, h, c0:c0 + ncc][:, None, :]
                            .broadcast_to([P, 64, ncc]),
                        )
                    part = smallp.tile([P, 64], F32, name="part")
                    nc.vector.tensor_reduce(
                        part[:], prod2[:],
                        axis=mybir.AxisListType.X, op=mybir.AluOpType.add,
                    )
                    # masked[cl=(par,e)] = partial[e] * (i%2==par)
                    mk = smallp.tile([P, P], BF16, name="mk")
                    nc.vector.tensor_mul(
                        mk[:].rearrange("p (q e) -> p q e", q=2),
                        part[:][:, None, :].broadcast_to([P, 2, 64]),
                        pmask[:].rearrange("p (q e) -> p q e", q=2),
                    )
                    masked.append(mk)
                for jh in range(2):
                    aps = pps.tile([P, 64], F32, name="pps")
                    nc.tensor.matmul(aps[:], masked[jh][:], sel[:],
                                     start=True, stop=False)
                    nc.tensor.matmul(aps[:], masked[2 + jh][:], sel[:],
                                     start=False, stop=True)
                    rb = bh * 16 + jh * 8
                    nc.vector.tensor_copy(
                        olTs[rb // P][:, :, rb % P:rb % P + 8],
                        aps[:].rearrange("p (g r) -> p g r", g=8),
                    )
                # ---- output projection for a finished row-chunk --------
                if bh % 8 == 7:
                    rc = bh // 8
                    for oh in range(2):
                        ps = gps.tile([P, 512], F32, name="gps")
                        for g in range(8):
                            nc.tensor.matmul(
                                ps[:],
                                olTs[rc][:, g, :],
                                wo[:, g, 512 * oh:512 * oh + 512],
                                start=(g == 0),
                                stop=(g == 7),
                            )
                        osb = prodp.tile([P, 512], F32, name="osb")
                        if with_bias:
                            nc.vector.tensor_add(
                                osb[:], ps[:], bo[:, 512 * oh:512 * oh + 512])
                        else:
                            nc.scalar.copy(osb[:], ps[:])
                        nc.sync.dma_start(
                            outd[128 * rc:128 * rc + 128,
                                 512 * oh:512 * oh + 512],
                            osb[:],
                        )

    return nc


_NC_CACHE = {}


def _get_nc(with_bias=True):
    if with_bias not in _NC_CACHE:
        _NC_CACHE[with_bias] = _build_nc(with_bias)
    return _NC_CACHE[with_bias]


def _host_prep(q, k, v, Wq, bq, Wk, bk, Wv, bv, Wo, bo):
    """Build the 8 per-core input maps (all device arrays bf16 except biases)."""
    q = np.asarray(q, np.float32)
    k = np.asarray(k, np.float32)
    v = np.asarray(v, np.float32)
    Wqs = np.asarray(Wq, np.float32) * 0.125  # fold 1/sqrt(hd)
    bqs = np.asarray(bq, np.float32) * 0.125

    def wprep(Wm):
        # [P, 8, D]: [f_local, f_chunk, o] of W.T
        WT = np.ascontiguousarray(np.asarray(Wm, np.float32).T)  # [f, o]
        return np.ascontiguousarray(
            WT.reshape(8, P, D).transpose(1, 0, 2)).astype(NPBF)

    wq_b = wprep(Wqs)
    wk_b = wprep(Wk)
    wv_b = wprep(Wv)
    wo_b = wprep(Wo)

    i_idx = np.arange(P)  # rows (j', i) packing for consts
    pmask = ((i_idx[:, None] % 16 % 2) == (np.arange(P)[None, :] // 64)).astype(NPBF)
    # rows of pmask are (j', i): parity = i % 2; cols (par, e): par = col // 64
    r_i = i_idx % 16
    r_j = i_idx // 16
    g_col = np.arange(64) // 8
    j_col = np.arange(64) % 8
    selm = ((r_j[:, None] == j_col[None, :]) &
            ((r_i[:, None] // 2) == g_col[None, :])).astype(NPBF)

    jj = np.arange(16)
    hh = np.arange(H)
    in_maps = []
    for c in range(NCORES):
        srows = hh[:, None] * 128 + c * 16 + jj[None, :]  # [h, j]

        def kvprep(x):
            xs = x[:, srows]  # [b, h, j, w, d]
            xs = xs.reshape(B, H, 16, 16, 2, D).transpose(0, 1, 4, 2, 3, 5)
            flat = xs.reshape(ROWS, D)  # rows (b,h,p,j,i)
            # kTb[f_local, bh_block, f_chunk, r'] where block rows r' in [0,512)
            xT = flat.T.astype(NPBF)  # [D, ROWS]
            xT = xT.reshape(8, P, NBH, 512)  # fc, f, blk, r
            return np.ascontiguousarray(xT.transpose(1, 2, 0, 3))  # f, blk, fc, r

        qc = q[:, c * 256:(c + 1) * 256, :].reshape(512, D)
        qT = qc.T.astype(NPBF).reshape(8, P, 512)
        qTb = np.ascontiguousarray(qT.transpose(1, 0, 2))  # [P, 8, 512]

        in_maps.append({
            "kTb": kvprep(k),
            "vTb": kvprep(v),
            "qTb": qTb,
            "wq": wq_b, "wk": wk_b, "wv": wv_b, "wo": wo_b,
            "bq": bqs, "bk": np.asarray(bk, np.float32),
            "bv": np.asarray(bv, np.float32), "bo": np.asarray(bo, np.float32),
            "pmask": pmask, "sel": selm,
        })
    return in_maps


def kernel(q, k, v, Wq, bq, Wk, bk, Wv, bv, Wo, bo, num_heads, _trace=False):
    assert int(num_heads) == H
    with_bias = any(
        np.any(np.asarray(x, np.float32)) for x in (bq, bk, bv, bo))
    nc = _get_nc(with_bias)
    in_maps = _host_prep(q, k, v, Wq, bq, Wk, bk, Wv, bv, Wo, bo)
    res = run_bass_kernel_spmd(nc, in_maps, core_ids=list(range(NCORES)),
                               trace=_trace)
    full = np.zeros((B, S, D), np.float32)
    for c in range(NCORES):
        oc = res.results[c]["out"].reshape(B, H, 16, D)
        for h in range(H):
            full[:, h * 128 + c * 16: h * 128 + c * 16 + 16, :] = oc[:, h]
    if _trace:
        kernel._last_exec_ns = res.exec_time_ns
        kernel._last_results = res
    return full



# revision 3
# speedup vs baseline: 1.4211x; 1.4211x over previous
"""Trainium2 Bass kernel for nn_MultiHeadLocalAttention (B=2,S=2048,W=32,D=1024,H=16).

Sharding: query-chunk parallel over 8 cores (see baseline docstring). Core c
needs exactly k/v rows {h*128 + c*16 + j}, a perfect 8-way split of the
dominant k/v projection GEMMs.

Perf structure (no-bias fast path):
 - K-projection contraction is split: the first NF8 128-feature chunks run as
   fp8(e4m3) DoubleRow matmuls (data at natural scale, weights pre-scaled x16
   on the host), the remaining chunks as bf16 with data pre-scaled x16
   (exact). Both accumulate into one PSUM group at 16x scale; the PSUM->SBUF
   cast on the Activation engine applies the 1/16 for free.
 - PSUM->SBUF kproj casts run on the Activation engine (otherwise idle).
 - V-phase attention reduces run on the Pool (gpsimd) engine.
 - Scores / attn-weighted products stay on the vector engine.
All remaining matmuls are bf16 (fp32 accumulate). 1/sqrt(hd) is folded into
Wq/bq on the host.
"""
import sys
import types

sys.path.insert(0, "/opt/trn_rl_repo")

import numpy as np
import ml_dtypes

import concourse.bass as bass
import concourse.mybir as mybir
import concourse.tile as tile
from concourse.bass_utils import run_bass_kernel_spmd

BF16 = mybir.dt.bfloat16
F32 = mybir.dt.float32
FP8 = mybir.dt.float8e4
NPBF = ml_dtypes.bfloat16
NPF8 = ml_dtypes.float8_e4m3

B, S, W, D, H = 2, 2048, 32, 1024, 16
P = 128
NCORES = 8
ROWS = B * H * 2 * 16 * 16  # 16384 k/v rows per core, order (b,h,p,j,i)
NBH = B * H  # 32 (b,h) blocks of 512 rows

NF8 = 6            # k-proj feature chunks in fp8 (even, 0..8)
NPAIR = NF8 // 2   # fp8 DoubleRow pair-instructions per 512-col block
NBF = 8 - NF8      # bf16 chunks (data pre-scaled x16)
KSC = 16.0         # common product scale of the k-proj PSUM group


class _TC(tile.TileContext):
    """Walrus in this env rejects instructions carrying >1 sem wait (its
    setupSyncWait has a single wait slot). Two fixes at context exit:
    1. split any multi-wait instruction: excess waits move to same-engine
       NOPs inserted immediately before it (per-engine stream order makes
       this equivalent);
    2. emit the exit-drain's waits as individual SP wait_ge instructions
       instead of on the Drain itself."""

    def _split_multi_waits(self):
        nc = self.nc
        eng = {
            mybir.EngineType.PE: nc.tensor,
            mybir.EngineType.DVE: nc.vector,
            mybir.EngineType.Activation: nc.scalar,
            mybir.EngineType.Pool: nc.gpsimd,
            mybir.EngineType.SP: nc.sync,
        }
        end_bb = nc.cur_bb.bb
        for f in nc.m.functions:
            for blk in f.blocks:
                snapshot = list(blk.instructions)
                inserts = {}
                created = []
                for idx, ins in enumerate(snapshot):
                    si = getattr(ins, "sync_info", None)
                    if si is None or not si.on_wait or len(si.on_wait) <= 1:
                        continue
                    waits = list(si.on_wait)
                    nops = []
                    for w in waits[:-1]:
                        assert w.wait_reg is None, "register wait can't be split"
                        b = eng[ins.engine].nop()  # appends to end_bb
                        b.ins.sync_info = mybir.SyncInfo(on_wait=[w], on_update=[])
                        nops.append(b.ins)
                        created.append(b.ins)
                    si.on_wait = waits[-1:]
                    inserts[idx] = nops
                if not inserts:
                    continue
                created_ids = {id(n) for n in created}
                # pull the freshly-appended nops back out of the end block
                end_bb.instructions = [
                    i for i in end_bb.instructions if id(i) not in created_ids
                ]
                out = []
                for idx, ins in enumerate(snapshot):
                    out.extend(inserts.get(idx, ()))
                    out.append(ins)
                blk.instructions = out

    def _drain_and_barrier(self, tick_clock, wait_clock):
        self._split_multi_waits()
        gc = tick_clock.global_clock
        for proc, sem in sorted(wait_clock.sems.allocated().items()):
            ticks = gc.peek_next(proc) - 1
            if ticks > 0:
                val = ticks * (16 if sem.name.startswith("DMA") else 1)
                self.nc.sync.wait_ge(sem, val)
        self.nc.sync.drain()
        self.nc.all_engine_barrier()
        popped = self.nc._tile_sem_poison_stack.pop()
        assert popped is self._sem_poison
        self.nc.clear_and_free_semaphores(list(self.sems.allocated().values()))
        self.nc.all_engine_barrier()


def _build_nc(with_bias=True):
    import contextlib

    nc = bass.Bass()
    if NPAIR:
        kT8d = nc.dram_tensor("kT8", [P, NBH, NPAIR, 2, 512], FP8,
                              kind="ExternalInput")
        wk8d = nc.dram_tensor("wk8", [P, NPAIR, 2, D], FP8,
                              kind="ExternalInput")
    if NBF:
        kTbd = nc.dram_tensor("kTb", [P, NBH, NBF, 512], BF16,
                              kind="ExternalInput")
        wk16d = nc.dram_tensor("wk16", [P, NBF, D], BF16,
                               kind="ExternalInput")
    vTb = nc.dram_tensor("vTb", [P, NBH, 8, 512], BF16, kind="ExternalInput")
    qTb = nc.dram_tensor("qTb", [P, 8, 512], BF16, kind="ExternalInput")
    wqd = nc.dram_tensor("wq", [P, 8, D], BF16, kind="ExternalInput")
    wvd = nc.dram_tensor("wv", [P, 8, D], BF16, kind="ExternalInput")
    wod = nc.dram_tensor("wo", [P, 8, D], BF16, kind="ExternalInput")
    bqd = nc.dram_tensor("bq", [D], F32, kind="ExternalInput")
    bkd = nc.dram_tensor("bk", [D], F32, kind="ExternalInput")
    bvd = nc.dram_tensor("bv", [D], F32, kind="ExternalInput")
    bod = nc.dram_tensor("bo", [D], F32, kind="ExternalInput")
    pmaskd = nc.dram_tensor("pmask", [P, P], BF16, kind="ExternalInput")
    seld = nc.dram_tensor("sel", [P, 64], BF16, kind="ExternalInput")
    outd = nc.dram_tensor("out", [512, D], F32, kind="ExternalOutput")

    with _TC(nc) as tc, contextlib.ExitStack() as ex:
        wpool = ex.enter_context(tc.tile_pool(name="wts", bufs=1))
        persist = ex.enter_context(tc.tile_pool(name="persist", bufs=1))
        blk = ex.enter_context(tc.tile_pool(name="blk", bufs=3))
        actp = ex.enter_context(tc.tile_pool(name="act", bufs=10))
        prodp = ex.enter_context(tc.tile_pool(name="prod", bufs=4))
        smallp = ex.enter_context(tc.tile_pool(name="small", bufs=10))
        gps = ex.enter_context(tc.tile_pool(name="gps", bufs=6, space="PSUM"))
        pps = ex.enter_context(tc.tile_pool(name="pps", bufs=2, space="PSUM"))

        # ---- constants -------------------------------------------------
        def load_w(dram_t, name, shape, dt):
            t = wpool.tile(shape, dt, name=name)
            nc.sync.dma_start(t[:], dram_t[:])
            return t

        wq = load_w(wqd, "wq", [P, 8, D], BF16)
        if NPAIR:
            wk8 = load_w(wk8d, "wk8", [P, NPAIR, 2, D], FP8)
        if NBF:
            wk16 = load_w(wk16d, "wk16", [P, NBF, D], BF16)
        wv = load_w(wvd, "wv", [P, 8, D], BF16)
        wo = load_w(wod, "wo", [P, 8, D], BF16)

        def load_b(dram_t, name):
            t = wpool.tile([P, D], F32, name=name)
            nc.sync.dma_start(t[:], dram_t[:].partition_broadcast(P))
            return t

        if with_bias:
            bq = load_b(bqd, "bq")
            bk = load_b(bkd, "bk")
            bv = load_b(bvd, "bv")
            bo = load_b(bod, "bo")

        pmask = wpool.tile([P, P], BF16, name="pmask")
        nc.sync.dma_start(pmask[:], pmaskd[:])
        sel = wpool.tile([P, 64], BF16, name="sel")
        nc.sync.dma_start(sel[:], seld[:])

        # ---- q projection (bf16, natural scale) ------------------------
        qts = persist.tile([P, 8, 512], BF16, name="qts")
        nc.sync.dma_start(qts[:], qTb[:])
        qp = persist.tile([P, 4, D], BF16, name="qp")  # u-chunks (b, jh)
        for t in range(4):
            for oh in range(2):
                ps = gps.tile([P, 512], F32, name="gps")
                for fc in range(8):
                    nc.tensor.matmul(
                        ps[:],
                        qts[:, fc, 128 * t:128 * t + 128],
                        wq[:, fc, 512 * oh:512 * oh + 512],
                        start=(fc == 0),
                        stop=(fc == 7),
                    )
                if with_bias:
                    nc.vector.tensor_add(
                        qp[:, t, 512 * oh:512 * oh + 512],
                        ps[:],
                        bq[:, 512 * oh:512 * oh + 512],
                    )
                else:
                    nc.scalar.copy(qp[:, t, 512 * oh:512 * oh + 512], ps[:])

        # ---- score planes / attn planes (persistent) -------------------
        planes = [[persist.tile([P, H, 16], F32, name=f"pl_{b}_{t}")
                   for t in range(4)] for b in range(B)]
        attns = [[persist.tile([P, H, 16], BF16, name=f"at_{b}_{t}")
                  for t in range(4)] for b in range(B)]
        # transposed out_local, chunked by contraction block g and row-chunk:
        # olT[rc][cl, g, r] = out_local[row 128*rc + r, c = 128*g + cl]
        olTs = [persist.tile([P, 8, P], BF16, name=f"olT{rc}")
                for rc in range(4)]

        # ---- K phase: split fp8/bf16 projection + scores ---------------
        for b in range(B):
            for h in range(H):
                bh = b * H + h
                if NPAIR:
                    k8t = blk.tile([P, NPAIR, 2, 512], FP8, name="k8")
                    nc.sync.dma_start(k8t[:], kT8d[:, bh])
                if NBF:
                    k16t = blk.tile([P, NBF, 512], BF16, name="k16")
                    nc.sync.dma_start(k16t[:], kTbd[:, bh])
                for t in range(4):
                    kv = actp.tile([P, D], BF16, name="kv")
                    for oh in range(2):
                        ps = gps.tile([P, 512], F32, name="gps")
                        nmm = NPAIR + NBF
                        i = 0
                        for fcp in range(NPAIR):
                            nc.tensor.matmul(
                                ps[:],
                                k8t[:, fcp, :, 128 * t:128 * t + 128],
                                wk8[:, fcp, :, 512 * oh:512 * oh + 512],
                                start=(i == 0),
                                stop=(i == nmm - 1),
                                perf_mode=mybir.MatmulPerfMode.DoubleRow,
                            )
                            i += 1
                        for fb in range(NBF):
                            nc.tensor.matmul(
                                ps[:],
                                k16t[:, fb, 128 * t:128 * t + 128],
                                wk16[:, fb, 512 * oh:512 * oh + 512],
                                start=(i == 0),
                                stop=(i == nmm - 1),
                            )
                            i += 1
                        dst = kv[:, 512 * oh:512 * oh + 512]
                        if with_bias:
                            nc.vector.scalar_tensor_tensor(
                                dst, ps[:], 1.0 / KSC,
                                bk[:, 512 * oh:512 * oh + 512],
                                op0=mybir.AluOpType.mult,
                                op1=mybir.AluOpType.add,
                            )
                        else:
                            nc.scalar.mul(dst, ps[:], 1.0 / KSC)
                    jh = t % 2
                    qsm = (qp[:, 2 * b + jh, 64 * h:64 * h + 64][:, None, :])
                    prod = prodp.tile([P, 16, 64], BF16, name="prod")
                    nc.vector.tensor_mul(
                        prod[:],
                        kv[:].rearrange("p (c e) -> p c e", c=16),
                        qsm.broadcast_to([P, 16, 64]),
                    )
                    nc.vector.tensor_reduce(
                        planes[b][t][:, h, :],
                        prod[:],
                        axis=mybir.AxisListType.X,
                        op=mybir.AluOpType.add,
                    )

        # ---- softmax over heads ----------------------------------------
        for b in range(B):
            for t in range(4):
                pl = planes[b][t]
                mx = smallp.tile([P, 16], F32, name="mx")
                nc.vector.tensor_reduce(
                    mx[:], pl[:].rearrange("p h c -> p c h"),
                    axis=mybir.AxisListType.X, op=mybir.AluOpType.max,
                )
                sub = prodp.tile([P, H, 16], F32, name="sm")
                nc.vector.tensor_sub(
                    sub[:], pl[:],
                    mx[:][:, None, :].broadcast_to([P, H, 16]),
                )
                epl = prodp.tile([P, H, 16], F32, name="ep")
                nc.scalar.activation(
                    epl[:], sub[:], mybir.ActivationFunctionType.Exp,
                )
                z = smallp.tile([P, 16], F32, name="z")
                nc.vector.tensor_reduce(
                    z[:], epl[:].rearrange("p h c -> p c h"),
                    axis=mybir.AxisListType.X, op=mybir.AluOpType.add,
                )
                rz = smallp.tile([P, 16], F32, name="rz")
                nc.vector.reciprocal(rz[:], z[:])
                nc.vector.tensor_mul(
                    attns[b][t][:], epl[:],
                    rz[:][:, None, :].broadcast_to([P, H, 16]),
                )

        # ---- V phase: bf16 projection + weighted sum + transpose-repack
        for b in range(B):
            for h in range(H):
                bh = b * H + h
                ts_ = blk.tile([P, 8, 512], BF16, name="vblk")
                nc.sync.dma_start(ts_[:], vTb[:, bh])
                vps = []
                for t in range(4):
                    if with_bias:
                        vv = actp.tile([P, D], BF16, name="vv")
                    halves = []
                    for oh in range(2):
                        ps = gps.tile([P, 512], F32, name="gps")
                        for fc in range(8):
                            nc.tensor.matmul(
                                ps[:],
                                ts_[:, fc, 128 * t:128 * t + 128],
                                wv[:, fc, 512 * oh:512 * oh + 512],
                                start=(fc == 0),
                                stop=(fc == 7),
                            )
                        if with_bias:
                            nc.vector.tensor_add(
                                vv[:, 512 * oh:512 * oh + 512],
                                ps[:],
                                bv[:, 512 * oh:512 * oh + 512],
                            )
                        else:
                            halves.append(ps)
                    if with_bias:
                        vps.append([(vv[:].rearrange("p (c e) -> p c e", c=16),
                                     0, 16)])
                    else:
                        vps.append([
                            (halves[0][:].rearrange("p (c e) -> p c e", c=8),
                             0, 8),
                            (halves[1][:].rearrange("p (c e) -> p c e", c=8),
                             8, 8),
                        ])
                masked = []
                for t in range(4):
                    prod2 = prodp.tile([P, 64, 16], BF16, name="pr2")
                    for src, c0, ncc in vps[t]:
                        nc.vector.tensor_mul(
                            prod2[:, :, c0:c0 + ncc],
                            src.rearrange("p c e -> p e c"),
                            attns[b][t][:, h, c0:c0 + ncc][:, None, :]
                            .broadcast_to([P, 64, ncc]),
                        )
                    part = smallp.tile([P, 64], F32, name="part")
                    nc.vector.tensor_reduce(
                        part[:], prod2[:],
                        axis=mybir.AxisListType.X, op=mybir.AluOpType.add,
                    )
                    # masked[cl=(par,e)] = partial[e] * (i%2==par)
                    mk = smallp.tile([P, P], BF16, name="mk")
                    nc.vector.tensor_mul(
                        mk[:].rearrange("p (q e) -> p q e", q=2),
                        part[:][:, None, :].broadcast_to([P, 2, 64]),
                        pmask[:].rearrange("p (q e) -> p q e", q=2),
                    )
                    masked.append(mk)
                for jh in range(2):
                    aps = pps.tile([P, 64], F32, name="pps")
                    nc.tensor.matmul(aps[:], masked[jh][:], sel[:],
                                     start=True, stop=False)
                    nc.tensor.matmul(aps[:], masked[2 + jh][:], sel[:],
                                     start=False, stop=True)
                    rb = bh * 16 + jh * 8
                    nc.vector.tensor_copy(
                        olTs[rb // P][:, :, rb % P:rb % P + 8],
                        aps[:].rearrange("p (g r) -> p g r", g=8),
                    )
                # ---- output projection for a finished row-chunk --------
                if bh % 8 == 7:
                    rc = bh // 8
                    for oh in range(2):
                        ps = gps.tile([P, 512], F32, name="gps")
                        for g in range(8):
                            nc.tensor.matmul(
                                ps[:],
                                olTs[rc][:, g, :],
                                wo[:, g, 512 * oh:512 * oh + 512],
                                start=(g == 0),
                                stop=(g == 7),
                            )
                        osb = prodp.tile([P, 512], F32, name="osb")
                        if with_bias:
                            nc.vector.tensor_add(
                                osb[:], ps[:], bo[:, 512 * oh:512 * oh + 512])
                        else:
                            nc.scalar.copy(osb[:], ps[:])
                        nc.sync.dma_start(
                            outd[128 * rc:128 * rc + 128,
                                 512 * oh:512 * oh + 512],
                            osb[:],
                        )

    return nc


_NC_CACHE = {}


def _get_nc(with_bias=True):
    if with_bias not in _NC_CACHE:
        _NC_CACHE[with_bias] = _build_nc(with_bias)
    return _NC_CACHE[with_bias]


def _host_prep(q, k, v, Wq, bq, Wk, bk, Wv, bv, Wo, bo):
    """Build the 8 per-core input maps."""
    q = np.asarray(q, np.float32)
    k = np.asarray(k, np.float32)
    v = np.asarray(v, np.float32)
    Wqs = np.asarray(Wq, np.float32) * 0.125  # fold 1/sqrt(hd)
    bqs = np.asarray(bq, np.float32) * 0.125

    def wprep(Wm):
        # [P, 8, D]: [f_local, f_chunk, o] of W.T
        WT = np.ascontiguousarray(np.asarray(Wm, np.float32).T)  # [f, o]
        return np.ascontiguousarray(
            WT.reshape(8, P, D).transpose(1, 0, 2)).astype(NPBF)

    wq_b = wprep(Wqs)
    wv_b = wprep(Wv)
    wo_b = wprep(Wo)

    # k weights: fp8 chunks (scaled x16) + bf16 chunks (natural)
    WkT = np.ascontiguousarray(np.asarray(Wk, np.float32).T)  # [f, o]
    consts = {}
    if NPAIR:
        w8 = (KSC * WkT[:NF8 * P]).astype(NPF8)
        consts["wk8"] = np.ascontiguousarray(
            w8.reshape(NPAIR, 2, P, D).transpose(2, 0, 1, 3))
    if NBF:
        w16 = WkT[NF8 * P:].astype(NPBF)
        consts["wk16"] = np.ascontiguousarray(
            w16.reshape(NBF, P, D).transpose(1, 0, 2))

    i_idx = np.arange(P)  # rows (j', i) packing for consts
    pmask = ((i_idx[:, None] % 16 % 2) == (np.arange(P)[None, :] // 64)).astype(NPBF)
    r_i = i_idx % 16
    r_j = i_idx // 16
    g_col = np.arange(64) // 8
    j_col = np.arange(64) % 8
    selm = ((r_j[:, None] == j_col[None, :]) &
            ((r_i[:, None] // 2) == g_col[None, :])).astype(NPBF)

    jj = np.arange(16)
    hh = np.arange(H)
    in_maps = []
    for c in range(NCORES):
        srows = hh[:, None] * 128 + c * 16 + jj[None, :]  # [h, j]

        def kvprep_T(x):
            xs = x[:, srows]  # [b, h, j, w, d]
            xs = xs.reshape(B, H, 16, 16, 2, D).transpose(0, 1, 4, 2, 3, 5)
            flat = xs.reshape(ROWS, D)  # rows (b,h,p,j,i)
            return flat.T  # [D, ROWS] f32

        kT = kvprep_T(k)
        m = {}
        if NPAIR:
            k8 = kT[:NF8 * P].astype(NPF8)  # [NF8*P, ROWS]
            m["kT8"] = np.ascontiguousarray(
                k8.reshape(NPAIR, 2, P, NBH, 512).transpose(2, 3, 0, 1, 4))
        if NBF:
            k16 = (KSC * kT[NF8 * P:]).astype(NPBF)
            m["kTb"] = np.ascontiguousarray(
                k16.reshape(NBF, P, NBH, 512).transpose(1, 2, 0, 3))

        vT = kvprep_T(v).astype(NPBF)  # [D, ROWS]
        m["vTb"] = np.ascontiguousarray(
            vT.reshape(8, P, NBH, 512).transpose(1, 2, 0, 3))

        qc = q[:, c * 256:(c + 1) * 256, :].reshape(512, D)
        qT = qc.T.astype(NPBF).reshape(8, P, 512)
        m["qTb"] = np.ascontiguousarray(qT.transpose(1, 0, 2))  # [P, 8, 512]

        m.update({
            "wq": wq_b, "wv": wv_b, "wo": wo_b,
            "bq": bqs, "bk": np.asarray(bk, np.float32),
            "bv": np.asarray(bv, np.float32), "bo": np.asarray(bo, np.float32),
            "pmask": pmask, "sel": selm,
        })
        m.update(consts)
        in_maps.append(m)
    return in_maps


def kernel(q, k, v, Wq, bq, Wk, bk, Wv, bv, Wo, bo, num_heads, _trace=False):
    assert int(num_heads) == H
    with_bias = any(
        np.any(np.asarray(x, np.float32)) for x in (bq, bk, bv, bo))
    nc = _get_nc(with_bias)
    in_maps = _host_prep(q, k, v, Wq, bq, Wk, bk, Wv, bv, Wo, bo)
    res = run_bass_kernel_spmd(nc, in_maps, core_ids=list(range(NCORES)),
                               trace=_trace)
    full = np.zeros((B, S, D), np.float32)
    for c in range(NCORES):
        oc = res.results[c]["out"].reshape(B, H, 16, D)
        for h in range(H):
            full[:, h * 128 + c * 16: h * 128 + c * 16 + 16, :] = oc[:, h]
    if _trace:
        kernel._last_exec_ns = res.exec_time_ns
        kernel._last_results = res
    return full
